# revision 1
# baseline (speedup 1.0000x reference)
"""Trainium2 Bass kernel for nn_DRO_TOPK (margin-loss top-k + masked sim stats).

Strategy (8 NeuronCores, data-parallel over rows, symmetry-halved):
  - sim = X @ X.T is symmetric: every unordered pair {i, j} is covered once
    by the half-circle band d = (j - i) mod 4096 in [1, 2048]. Each core
    computes, for its 512 rows, a [128, 2176]-wide rectangle per row-tile
    (cols [a, a+2176) in core-local rotated coordinates, a = t*128) that
    covers each row's band plus <=128 junk cells/row (diag + mirror
    duplicates), which the host filters out by index.
  - Per-core inputs are column-ROTATED by c*512 so the band always sits at
    local cols [0, 2560) -> one uniform SPMD program; only 5 of 8 MB of
    X^T per core is ever touched.
  - On chip: w[i,j] = (notsame - 0.5) * sim in {-s/2, +s/2}; pair_loss =
    relu(MARGIN + 2*w), monotone in w. Device emits per-row top-8 of w
    (max8 + max_index) and Sign-bracketed zero-loss counts on the Scalar
    engine. Matmuls run as float32r (1 cyc/row at N>=256).
  - Host: drops junk by index (d outside [1,2048]), recomputes surviving
    candidate sims exactly in f64, takes top-10 unique pairs (x2 = the
    reference's top-20), and computes mean_pos/mean_neg/counts exactly in
    f64. Guards (top-8 sufficiency, zero-count bracket) trigger a full
    numpy fallback if the fast path cannot be proven exact.
"""

import os
import sys

import numpy as np

for _p in ('/opt/trn_rl_repo', '/root/.axon_site/_ro/trn_rl_repo'):
    if os.path.isdir(_p) and _p not in sys.path:
        sys.path.insert(0, _p)

N, D, NCORES = 4096, 512, 8
R = N // NCORES            # 512 rows per core
NT = R // 128              # 4 row-tiles per core
HB = N // 2                # 2048 half-circle band width
W_RECT = HB + 128          # 2176 rect width per row-tile
XCOLS = 3 * 128 + W_RECT   # 2560 cols of rotated X^T each core touches
KK = D // 128              # 4 contraction sub-tiles
MARGIN, BETA, TOPK = 0.5, 0.0, 20
ZTHR = -MARGIN / 2.0       # w <= ZTHR  <=>  pair_loss == 0
DELTA = 1e-3               # zero-count bracket width

_prog_cache = {}


def _round_f32r(a):
    """Round f32 array to float32r (RN to 11 mantissa bits), so the on-device
    f32r matmul consumes exactly these values via a non-casting DMA."""
    bits = a.astype(np.float32).view(np.uint32)
    rnd = ((bits.astype(np.uint64) + 0x800) >> 12 << 12).astype(np.uint32)
    return rnd.view(np.float32)


def _build_program():
    import concourse.bacc as bacc
    import concourse.mybir as mybir
    from concourse.tile import TileContext

    f32 = mybir.dt.float32
    f16 = mybir.dt.float16
    u32 = mybir.dt.uint32
    f32r = mybir.dt.float32r
    Alu = mybir.AluOpType
    Act = mybir.ActivationFunctionType

    nc = bacc.Bacc('TRN2', target_bir_lowering=False, debug=False)
    xtr_d = nc.dram_tensor('xtr', [KK, 128, XCOLS], f32r, kind='ExternalInput')
    tgr_d = nc.dram_tensor('tgr', [XCOLS], f16, kind='ExternalInput')
    tgf_d = nc.dram_tensor('tgf', [128, NT], f32, kind='ExternalInput')
    jmask_d = nc.dram_tensor('jmask', [128, W_RECT], f16, kind='ExternalInput')
    # fused output, device-native layout: per partition p:
    # [cand(t,j): 32 | zlo(t,half): 36:44 | zhi(t,half): 44:52]
    outp_d = nc.dram_tensor('outp', [128, 52], f32, kind='ExternalOutput')

    with TileContext(nc) as tc:
        with (
            tc.tile_pool(name='xts', bufs=1) as xts_pool,
            tc.tile_pool(name='tb', bufs=1) as tb_pool,
            tc.tile_pool(name='w', bufs=2) as w_pool,
            tc.tile_pool(name='mb', bufs=2) as mb_pool,
            tc.tile_pool(name='zs', bufs=1) as zs_pool,
            tc.tile_pool(name='small', bufs=1) as small_pool,
            tc.tile_pool(name='psb', bufs=3, space='PSUM') as psb_pool,
            tc.tile_pool(name='pst', bufs=2, space='PSUM') as pst_pool,
        ):
            # Rotated X^T in SBUF: 4 partition-tiles of [128, 2560] f32r,
            # each a single contiguous-per-partition DMA (fat descriptors).
            xts = [xts_pool.tile([128, XCOLS], f32r, tag=f'xt{kk}',
                                 name=f'xts{kk}') for kk in range(KK)]
            # Rotated targets (f16) broadcast to all 128 partitions.
            tb = tb_pool.tile([128, XCOLS], f16)
            nc.sync.dma_start(tb[:, :], tgr_d[:].unsqueeze(0).partition_broadcast(128))
            # Per-partition row targets (f32): tr[p, t] = target[t*128 + p].
            tr = small_pool.tile([128, NT], f32, tag='tr')
            nc.sync.dma_start(tr[:, :], tgf_d[:, :])
            # band mask J[p, x] = 1 iff 1 <= x - p <= 2047 (junk cells -> 0)
            jm = small_pool.tile([128, W_RECT], f16, tag='jm')
            nc.sync.dma_start(jm[:, :], jmask_d[:, :])

            for kk in range(KK):
                nc.sync.dma_start(xts[kk][:, :], xtr_d[kk, :, :])

            outt = small_pool.tile([128, 52], f32, tag='outt')
            bias_hi = small_pool.tile([128, 1], f32, tag='bias_hi')
            nc.vector.memset(bias_hi[:, :], -(ZTHR + DELTA))
            bias_lo = small_pool.tile([128, 1], f32, tag='bias_lo')
            nc.vector.memset(bias_lo[:, :], -(ZTHR - DELTA))

            for t in range(NT):
                a = t * 128
                # (notsame - 0.5) in fp16, then band-masked by J.
                mb0 = mb_pool.tile([128, W_RECT], f16, tag='mb0')
                nc.vector.tensor_scalar(mb0[:, :], tb[:, a:a + W_RECT],
                                        tr[:, t:t + 1], 0.5,
                                        Alu.not_equal, Alu.subtract)
                mb = mb_pool.tile([128, W_RECT], f16, tag='mb')
                nc.vector.tensor_tensor(mb[:, :], mb0[:, :], jm[:, :],
                                        op=Alu.mult)
                w = w_pool.tile([128, W_RECT], f32)
                # band pieces: 2x [128,1024] (2 PSUM banks) + 1x [128,128]
                for piece in range(2):
                    ps = psb_pool.tile([128, 1024], f32, name=f'psb{t}_{piece}',
                                       tag='psb')
                    for h in range(2):
                        o = a + piece * 1024 + h * 512
                        for kk in range(KK):
                            nc.tensor.matmul(ps[:, h * 512:(h + 1) * 512],
                                             xts[kk][:, a:a + 128],
                                             xts[kk][:, o:o + 512],
                                             start=(kk == 0), stop=(kk == KK - 1))
                    nc.vector.tensor_tensor(
                        w[:, piece * 1024:(piece + 1) * 1024],
                        mb[:, piece * 1024:(piece + 1) * 1024], ps[:, :],
                        op=Alu.mult)
                pt = pst_pool.tile([128, 128], f32, tag='pst')
                o = a + 2048
                for kk in range(KK):
                    nc.tensor.matmul(pt[:, :], xts[kk][:, a:a + 128],
                                     xts[kk][:, o:o + 128],
                                     start=(kk == 0), stop=(kk == KK - 1))
                nc.vector.tensor_tensor(w[:, 2048:2176], mb[:, 2048:2176],
                                        pt[:, :], op=Alu.mult)
                # Per-row top-8 candidates of w.
                nc.vector.max(outt[:, t * 8:(t + 1) * 8], w[:, :])
                # Sign-bracketed zero-loss counts on ACT, in halves.
                for hv in range(2):
                    wh = w[:, hv * 1088:(hv + 1) * 1088]
                    z1 = zs_pool.tile([128, 1088], f32, tag='z1')
                    nc.scalar.activation(z1[:, :], wh, Act.Sign,
                                         bias=bias_hi[:, :],
                                         accum_out=outt[:, 44 + t * 2 + hv:
                                                        45 + t * 2 + hv])
                    z2 = zs_pool.tile([128, 1088], f32, tag='z2')
                    nc.scalar.activation(z2[:, :], wh, Act.Sign,
                                         bias=bias_lo[:, :],
                                         accum_out=outt[:, 36 + t * 2 + hv:
                                                        37 + t * 2 + hv])

            for q in range(4):
                nc.sync.dma_start(outp_d[q * 32:(q + 1) * 32, :],
                                  outt[q * 32:(q + 1) * 32, :])

    nc.compile()
    return nc


def _numpy_fallback(x, t):
    """Faithful f32 numpy recompute of the full reference (safety net)."""
    sim = x @ x.T
    same = t[:, None] == t[None, :]
    eye = np.eye(N, dtype=bool)
    pos = same & ~eye
    neg = ~same
    pos_l = np.maximum(MARGIN + BETA - sim, 0.0).astype(np.float32)
    neg_l = np.maximum(MARGIN + sim - BETA, 0.0).astype(np.float32)
    valid = pos | neg
    pair = np.where(pos, pos_l, neg_l)
    zeros = int((valid & (pair == 0.0)).sum())
    masked = np.where(valid, pair, -np.inf).ravel()
    top = np.sort(masked)[-TOPK:]
    loss = np.float32(top.astype(np.float64).mean())
    mean_pos = np.float32(sim[pos].astype(np.float64).sum() / pos.sum())
    mean_neg = np.float32(sim[neg].astype(np.float64).sum() / neg.sum())
    return loss, np.int32(zeros), mean_pos, mean_neg


def kernel(**inputs):
    from concourse.bass_utils import run_bass_kernel_spmd

    x = np.ascontiguousarray(inputs['inputs'].astype(np.float32, copy=False))
    t = np.asarray(inputs['targets'])
    t_i = t.astype(np.int64)
    t16 = t.astype(np.float16)
    t32 = t.astype(np.float32)

    if 'nc' not in _prog_cache:
        _prog_cache['nc'] = _build_program()
        pj, xj = np.arange(128)[:, None], np.arange(W_RECT)[None, :]
        dj = xj - pj
        _prog_cache['jmask'] = ((dj >= 1) & (dj <= HB - 1)).astype(np.float16)
    nc = _prog_cache['nc']
    jmask = _prog_cache['jmask']

    xt = _round_f32r(np.ascontiguousarray(x.T))          # [D, N] f32r values
    xt2 = np.concatenate([xt, xt[:, :XCOLS - N]], axis=1)   # wrap for rotation
    t16w = np.concatenate([t16, t16[:XCOLS - N]])
    in_maps = []
    for c in range(NCORES):
        sh = c * R
        in_maps.append({
            'xtr': np.ascontiguousarray(
                xt2[:, sh:sh + XCOLS].reshape(KK, 128, XCOLS)),
            'tgr': np.ascontiguousarray(t16w[sh:sh + XCOLS]),
            'tgf': np.ascontiguousarray(
                t32[sh:sh + R].reshape(NT, 128).T),
            'jmask': jmask,
        })

    res = run_bass_kernel_spmd(nc, in_maps, core_ids=list(range(NCORES)))

    cands, zlos, zhis = [], [], []
    for r in res.results:
        o = r['outp']                                   # [128, 52]
        cands.append(o[:, 0:32].reshape(128, NT, 8).transpose(1, 0, 2)
                     .reshape(R, 8))
        zlos.append(o[:, 36:44].reshape(128, NT, 2).sum(axis=2)
                    .T.reshape(R))
        zhis.append(o[:, 44:52].reshape(128, NT, 2).sum(axis=2)
                    .T.reshape(R))
    cand = np.concatenate(cands, axis=0)                # [N, 8]
    zsum_lo = np.concatenate(zlos)                      # [N]
    zsum_hi = np.concatenate(zhis)

    x64 = x.astype(np.float64)
    # ---- antipodal (d = 2048) pairs: fixed index set, exact on host ----
    ai = np.arange(HB)
    s_ant = np.einsum('nd,nd->n', x64[ai], x64[ai + HB])
    w_ant = 0.5 * np.where(t_i[ai] == t_i[ai + HB], -1.0, 1.0) * s_ant

    # ---- top-10 unique pairs (x2 = reference top-20) ----
    # device candidates cover d in [1, 2047] once each; junk cells read 0.
    merged = np.concatenate([cand.ravel(), w_ant])
    top10 = np.sort(merged)[-(TOPK // 2):]
    t10 = top10[0]
    sufficiency_ok = bool((cand[:, 7] <= t10).all()) and t10 > 1e-6

    # ---- zero count guards (device counts in-band cells only) ----
    c_lo = (W_RECT - zsum_lo) / 2.0
    c_hi = (W_RECT - zsum_hi) / 2.0
    zeros_ok = (np.all(c_lo == 0.0) and np.all(c_hi == 0.0)
                and not np.any(w_ant <= ZTHR + DELTA))
    if not (sufficiency_ok and zeros_ok):
        return _numpy_fallback(x, t_i)
    num_zeros = 0

    top20 = np.repeat(top10[::-1], 2)
    loss = np.float32(np.maximum(MARGIN + 2.0 * top20.astype(np.float64), 0.0).mean())

    # ---- exact f64 stats on host ----
    G = np.zeros((int(t_i.max()) + 1, D), dtype=np.float64)
    np.add.at(G, t_i, x64)
    cls_sq = float((G * G).sum())
    diag_sq = float((x64 * x64).sum())
    cnt = np.bincount(t_i)
    pos_cnt = int((cnt.astype(np.int64) * (cnt - 1)).sum())
    neg_cnt = N * N - int((cnt.astype(np.int64) ** 2).sum())
    tot = x64.sum(axis=0)
    total_sq = float(tot @ tot)
    mean_pos = np.float32((cls_sq - diag_sq) / pos_cnt)
    mean_neg = np.float32((total_sq - cls_sq) / neg_cnt)

    return loss, np.int32(num_zeros), mean_pos, mean_neg



# revision 5
# speedup vs baseline: 1.4729x; 1.4729x over previous
"""Trainium2 Bass kernel for nn_DRO_TOPK (margin-loss top-k + masked sim stats).

Strategy (8 NeuronCores, data-parallel over rows, symmetry-halved, maskless):
  - sim = X @ X.T is symmetric: every unordered pair {i, j} is covered once
    by the half-circle band d = (j - i) mod 4096 in [1, 2048]. Each core
    handles 512 rows as 4 row-tiles of 128; per tile it computes a
    [128, 2048] PSUM rect (cols [a, a+2048) in core-local rotated coords,
    a = t*128) with a bf16 matmul, but only consumes offsets [128, 2048):
    every cell there is a valid, unique cross-class-or-not pair
    (d = x - p in [1, 2047]) -- NO masks, mask DMAs, or mask multiplies.
  - The two junk/remainder triangles per tile (corner: d in [1, 127-p];
    right: d in [2048-p, 2048]) are tiny (128x128 blocks) and are computed
    EXACTLY on the host in f64 (64 small GEMMs), as are all same-class
    (positive) pairs via class buckets.
  - Device emits, per row, top-8 of the raw band sims (DVE max8 straight
    from PSUM) and a Sign-accumulator zero-loss certificate (no band cell
    <= -0.45) on the Scalar engine.
  - Inputs stream as bf16 column-chunks issued from 4 different sequencers
    so matmuls start while later chunks are still in flight.
  - Host merges: device neg-candidates (bf16-accurate), exact host
    triangle cells, exact positive-pair losses; margin guards (sufficiency,
    same-class pollution, zero certificates) trigger a full numpy fallback
    if the fast path cannot be proven correct.
"""

import os
import sys

import numpy as np

for _p in ('/opt/trn_rl_repo', '/root/.axon_site/_ro/trn_rl_repo'):
    if os.path.isdir(_p) and _p not in sys.path:
        sys.path.insert(0, _p)

N, D, NCORES = 4096, 512, 8
R = N // NCORES            # 512 rows per core
NT = R // 128              # 4 row-tiles per core
RECT = 2048                # psum rect width per row-tile
LO = 128                   # offsets [LO, RECT) are consumed -> 1920 cells/row
WB = RECT - LO             # 1920
XCOLS = 3 * 128 + RECT     # 2432 cols of rotated X^T each core touches
KK = D // 128              # 4 contraction sub-tiles
MARGIN, BETA, TOPK = 0.5, 0.0, 20
ZTHR = 0.45                # no-zero certificate: all band sims > -ZTHR
EPS = 5e-3                 # bf16 matmul error envelope for the guards

_prog_cache = {}


def _build_program():
    import concourse.bacc as bacc
    import concourse.mybir as mybir
    from concourse.tile import TileContext

    f32 = mybir.dt.float32
    bf16 = mybir.dt.bfloat16
    Act = mybir.ActivationFunctionType

    nc = bacc.Bacc('TRN2', target_bir_lowering=False, debug=False)
    xtr_d = nc.dram_tensor('xtr', [KK, 128, XCOLS], bf16, kind='ExternalInput')
    # per partition p: [cand(t,j): 0:32 | sign-accum(t): 32:36]
    outp_d = nc.dram_tensor('outp', [128, 36], f32, kind='ExternalOutput')

    with TileContext(nc) as tc:
        with (
            tc.tile_pool(name='xts', bufs=1) as xts_pool,
            tc.tile_pool(name='zs', bufs=1) as zs_pool,
            tc.tile_pool(name='small', bufs=1) as small_pool,
            tc.tile_pool(name='ps', bufs=2, space='PSUM') as ps_pool,
        ):
            # Rotated X^T in SBUF: 4 contraction sub-tiles of [128, 2432]
            # bf16, streamed in as 512-col chunks. Issue the DGE configs from
            # 4 different sequencers so all of chunk 0 is in flight fast.
            xts = [xts_pool.tile([128, XCOLS], bf16, tag=f'xt{kk}',
                                 name=f'xts{kk}') for kk in range(KK)]
            issuers = [nc.gpsimd, nc.sync, nc.scalar, nc.gpsimd]
            bounds = [0, 512, 1024, 1536, 2048, XCOLS]
            for ci in range(len(bounds) - 1):
                c0, c1 = bounds[ci], bounds[ci + 1]
                for kk in range(KK):
                    issuers[kk].dma_start(xts[kk][:, c0:c1],
                                          xtr_d[kk, :, c0:c1])

            outt = small_pool.tile([128, 36], f32, tag='outt')
            zdump = zs_pool.tile([128, WB], bf16, tag='zdump')
            zbias = small_pool.tile([128, 1], f32, tag='zbias')
            nc.vector.memset(zbias[:, :], ZTHR)

            for t in range(NT):
                a = t * 128
                ps = ps_pool.tile([128, RECT], f32, tag='ps', name=f'ps{t}')
                for h in range(4):
                    o = a + h * 512
                    for kk in range(KK):
                        nc.tensor.matmul(ps[:, h * 512:(h + 1) * 512],
                                         xts[kk][:, a:a + 128],
                                         xts[kk][:, o:o + 512],
                                         start=(kk == 0), stop=(kk == KK - 1))
                # Per-row top-8 raw band sims, straight from PSUM.
                nc.vector.max(outt[:, t * 8:(t + 1) * 8], ps[:, LO:RECT])
                # Zero-loss certificate: accum == WB iff no cell <= -ZTHR.
                nc.scalar.activation(zdump[:, :], ps[:, LO:RECT], Act.Sign,
                                     bias=zbias[:, :],
                                     accum_out=outt[:, 32 + t:33 + t])

            nc.sync.dma_start(outp_d[:, :], outt[:, :])

    nc.compile()
    return nc


def _numpy_fallback(x, t):
    """Faithful f32 numpy recompute of the full reference (safety net)."""
    sim = x @ x.T
    same = t[:, None] == t[None, :]
    eye = np.eye(N, dtype=bool)
    pos = same & ~eye
    neg = ~same
    pos_l = np.maximum(MARGIN + BETA - sim, 0.0).astype(np.float32)
    neg_l = np.maximum(MARGIN + sim - BETA, 0.0).astype(np.float32)
    valid = pos | neg
    pair = np.where(pos, pos_l, neg_l)
    zeros = int((valid & (pair == 0.0)).sum())
    masked = np.where(valid, pair, -np.inf).ravel()
    top = np.sort(masked)[-TOPK:]
    loss = np.float32(top.astype(np.float64).mean())
    mean_pos = np.float32(sim[pos].astype(np.float64).sum() / pos.sum())
    mean_neg = np.float32(sim[neg].astype(np.float64).sum() / neg.sum())
    return loss, np.int32(zeros), mean_pos, mean_neg


def kernel(**inputs):
    import ml_dtypes
    from concourse.bass_utils import run_bass_kernel_spmd

    x = np.ascontiguousarray(inputs['inputs'].astype(np.float32, copy=False))
    t = np.asarray(inputs['targets'])
    t_i = t.astype(np.int64)

    if 'nc' not in _prog_cache:
        _prog_cache['nc'] = _build_program()
    nc = _prog_cache['nc']

    xb = x.astype(ml_dtypes.bfloat16)                   # RNE rounding
    xt = np.ascontiguousarray(xb.T)                     # [D, N] bf16
    xt2 = np.concatenate([xt, xt[:, :XCOLS]], axis=1)   # wrap for rotation
    in_maps = []
    for c in range(NCORES):
        sh = c * R
        in_maps.append({
            'xtr': np.ascontiguousarray(
                xt2[:, sh:sh + XCOLS].reshape(KK, 128, XCOLS)),
        })

    res = run_bass_kernel_spmd(nc, in_maps, core_ids=list(range(NCORES)))

    cands, accs = [], []
    for r in res.results:
        o = r['outp']                                   # [128, 36]
        cands.append(o[:, 0:32].reshape(128, NT, 8).transpose(1, 0, 2)
                     .reshape(R, 8))
        accs.append(o[:, 32:36].T.reshape(R))
    cand = np.concatenate(cands, axis=0)                # [N, 8] band sims
    acc = np.concatenate(accs)                          # [N] sign accums

    x64 = x.astype(np.float64)

    # ---- exact host triangles: 32 corner + 32 right [128,128] blocks ----
    Xb = x64.reshape(32, 128, D)
    Xs = np.roll(x64, -RECT, axis=0).reshape(32, 128, D)
    CA = Xb @ Xb.transpose(0, 2, 1)                     # corner blocks
    RB = Xb @ Xs.transpose(0, 2, 1)                     # right blocks
    tb = t_i.reshape(32, 128)
    ts = np.roll(t_i, -RECT).reshape(32, 128)
    iu0, iu1 = np.triu_indices(128, 1)
    il0, il1 = np.tril_indices(128, -1)
    corner_s = CA[:, iu0, iu1].ravel()
    corner_same = (tb[:, iu0] == tb[:, iu1]).ravel()
    right_s = RB[:, il0, il1].ravel()
    right_same = (tb[:, il0] == ts[:, il1]).ravel()
    anti_s = RB[:16].diagonal(axis1=1, axis2=2).ravel()
    anti_same = (tb[:16] == ts[:16]).ravel()
    host_neg = np.concatenate([corner_s[~corner_same], right_s[~right_same],
                               anti_s[~anti_same]])
    host_cells = np.concatenate([corner_s, right_s, anti_s])

    # ---- all same-class (positive) pairs exactly, via class buckets ----
    order = np.argsort(t_i, kind='stable')
    ts_sorted = t_i[order]
    starts = np.flatnonzero(np.r_[True, ts_sorted[1:] != ts_sorted[:-1]])
    ends = np.r_[starts[1:], N]
    pos_sims = []
    for s0, s1 in zip(starts, ends):
        if s1 - s0 < 2:
            continue
        idx = order[s0:s1]
        S = x64[idx] @ x64[idx].T
        pos_sims.append(S[np.triu_indices(s1 - s0, 1)])
    pos_sims = (np.concatenate(pos_sims) if pos_sims
                else np.empty(0, np.float64))
    max_same = pos_sims.max() if pos_sims.size else -np.inf

    # ---- merge candidate losses, take top-10 unique pairs ----
    merged = np.concatenate([MARGIN + cand.ravel(),     # device neg cands
                             MARGIN + host_neg,         # exact host neg cells
                             MARGIN - pos_sims])        # exact pos pairs
    top10 = np.sort(merged)[-(TOPK // 2):]
    T = top10[0]

    # ---- guards: prove the fast path exact, else fall back ----
    ok = (
        bool(np.all(acc == float(WB)))                  # no band cell <= -0.45
        and MARGIN + cand[:, 7].max() + EPS < T         # top-8/row sufficient
        and MARGIN + max_same + EPS < T                 # no same-class leak
        and host_cells.min() > -ZTHR                    # host cells zero-free
        and (not pos_sims.size or max_same < ZTHR)
        and T > MARGIN + 0.05                           # sane top values
    )
    if not ok:
        return _numpy_fallback(x, t_i)

    loss = np.float32(top10.mean())
    num_zeros = 0

    # ---- exact f64 stats on host ----
    G = np.zeros((int(t_i.max()) + 1, D), dtype=np.float64)
    np.add.at(G, t_i, x64)
    cls_sq = float((G * G).sum())
    diag_sq = float((x64 * x64).sum())
    cnt = np.bincount(t_i)
    pos_cnt = int((cnt.astype(np.int64) * (cnt - 1)).sum())
    neg_cnt = N * N - int((cnt.astype(np.int64) ** 2).sum())
    tot = x64.sum(axis=0)
    total_sq = float(tot @ tot)
    mean_pos = np.float32((cls_sq - diag_sq) / pos_cnt)
    mean_neg = np.float32((total_sq - cls_sq) / neg_cnt)

    return loss, np.int32(num_zeros), mean_pos, mean_neg
